# revision 13
# baseline (speedup 1.0000x reference)
"""TRN2 Bass kernel for nn_ComboFwdVecComp (B=4, S=512, C=V=128).

out[b,i,j,v] = tanh( sum_c ctx[b,i,c]*ctx[b,j,c]*Wm[v,c]            (M)
                     + ctx[b,i,:] @ (W2-Wd).T                       (Bi, i-dep)
                     + ctx[b,j,:] @ (W1+Wd).T + (b1+b2+bm+bd) )     (Aj, j-dep)

Sharding: core k handles b = k//2, j in [ (k%2)*256, +256 ), ALL 512 i's.
Shard out (S, 256, V) fp16; host casts to f32 and scatters on the j axis.

v2 design (vs the 292us f32 baseline):
- fp16 output store halves HBM write traffic (32 MiB/core); rel-err budget
  2e-2 >> fp16 ulp. PSUM partition dim = i so each drained half [128 i,
  16 j, 128 v] maps to (j,v)-contiguous HBM: 2-dim DMAs, 4 KiB runs.
- all matmuls in fp16 (1 cyc/row like f32r, tf32-class mantissa, no
  rounding-producer rule: host supplies fp16, DVE prep emits fp16).
- Delta-matmul chain kills the K=1 bias-replication redundancy: per
  (jblock, bank) PSUM accumulates  bias(arow) + ctx_i0 . rhs'  and is
  drained; then lhsT = fp16(ctx_ic{k} - ctx_ic{k-1}) matmuls morph it in
  place to chunk k's pre-tanh, re-drained, k = 1..3. Bias rows are paid
  once per 4 i-chunks: PE rows 8jb*8banks*(512 + 4*512) ~ 164k ~ 117us,
  below the ACT tanh floor (64 drains * ~1.97us = 126us).
- rhs'[c,(j,v)] = WmT[c,v]*ctx[b,j,c] + W2dT[c,v] folds Bi into the main
  matmul; Aj + biases ride the K=1 ones-matmul from strip-packed arow.
"""

import sys
import types
from contextlib import ExitStack

import numpy as np

import concourse.bass as bass
import concourse.mybir as mybir
import concourse.tile as tile
from concourse import bacc
from concourse.bass_utils import run_bass_kernel_spmd

B, S, C, V = 4, 512, 128, 128
NCORES = 8
NJ = 256          # j's per core
NJB = 8           # j-blocks of 32 j's
NQ = NJ // 4      # j-quads per core (64)
NLEV = 4          # i-chunks of 128 (level 0 + 3 deltas)

_F32 = mybir.dt.float32
_F16 = mybir.dt.float16


def install_ntff_shim():
    """antenv.axon_hooks is absent on some images; shim it so trace=True works."""
    if "antenv.axon_hooks" in sys.modules:
        return
    try:
        from trn_agent_boot.trn_boot import _ntff_profile_via_ctypes
        hook = _ntff_profile_via_ctypes("/opt/axon/libaxon_pjrt.so")
    except Exception:
        hook = None
    mod = types.ModuleType("antenv.axon_hooks")
    mod.get_axon_ntff_profile_hook = lambda: hook
    mod.set_axon_ntff_profile_hook = lambda h: None
    sys.modules["antenv.axon_hooks"] = mod


def build_nc():
    nc = bacc.Bacc("TRN2", target_bir_lowering=False, debug=False,
                   num_swdge_queues=4)

    ctxjT_d = nc.dram_tensor("ctxjT", [C, NJ], _F32, kind="ExternalInput").ap()
    wmT_d = nc.dram_tensor("wmT", [C, V], _F32, kind="ExternalInput").ap()
    w2dT_d = nc.dram_tensor("w2dT", [C, V], _F32, kind="ExternalInput").ap()
    # [fp16(ctxT i 0:128) | delta1 | delta2 | delta3]
    ctxiD_d = nc.dram_tensor("ctxiD", [C, S], _F16, kind="ExternalInput").ap()
    # arow rows, strip-packed: quad q -> partition (q%4)*32, cols (q//4)*512
    arowp_d = nc.dram_tensor("arowp", [4, (NQ // 4) * 512], _F16, kind="ExternalInput").ap()
    out_d = nc.dram_tensor("out_shard", [S, NJ, V], _F16, kind="ExternalOutput").ap()

    with tile.TileContext(nc) as tc, ExitStack() as ctx:
        singles = ctx.enter_context(tc.tile_pool(name="singles", bufs=1))
        rhs_pool = ctx.enter_context(tc.tile_pool(name="rhs", bufs=8))
        tmp_pool = ctx.enter_context(tc.tile_pool(name="tmp", bufs=3))
        psum_pool = ctx.enter_context(tc.tile_pool(name="psum", bufs=1, space="PSUM"))
        out_pool = ctx.enter_context(tc.tile_pool(name="outs", bufs=16))

        # ---- load constants; arowp rows gate the first bias mms -> FIRST ----
        arowp_sb = singles.tile([97, (NQ // 4) * 512], _F16)
        for r in range(4):
            eng = nc.sync if r % 2 == 0 else nc.scalar
            eng.dma_start(out=arowp_sb[32 * r:32 * r + 1, :], in_=arowp_d[r:r + 1, :])
        ctxiD_sb = singles.tile([C, S], _F16)
        nc.sync.dma_start(out=ctxiD_sb, in_=ctxiD_d)
        ctxjT_sb = singles.tile([C, NJ], _F32)
        nc.scalar.dma_start(out=ctxjT_sb, in_=ctxjT_d)
        wmT_sb = singles.tile([C, V], _F32)
        nc.scalar.dma_start(out=wmT_sb, in_=wmT_d)
        w2dT_sb = singles.tile([C, V], _F32)
        nc.scalar.dma_start(out=w2dT_sb, in_=w2dT_d)

        ones_sb = singles.tile([97, 128], _F16)
        nc.vector.memset(ones_sb, 1.0)

        # broadcast APs for pair-wide (8 j's) prep: wmT/w2dT repeat over the
        # j dim (step 0), ctxj scalars repeat over the v dim (trailing step 0)
        wm_b8 = bass.AP(
            tensor=wmT_sb.tensor,
            offset=wmT_sb.offset,
            ap=[wmT_sb.ap[0], [0, 8], wmT_sb.ap[1]],
        )
        w2d_b8 = bass.AP(
            tensor=w2dT_sb.tensor,
            offset=w2dT_sb.offset,
            ap=[w2dT_sb.ap[0], [0, 8], w2dT_sb.ap[1]],
        )

        # one 8-bank psum megatile; bank b occupies [:, b*512:(b+1)*512]
        P = psum_pool.tile([128, 4096], _F32, name="mega")

        # 5 write streams: 1 HWDGE (sync), SWDGE ring 0 (plain dma_start on
        # gpsimd) and SWDGE rings 1-3 via kv_writeback (prep+trigger; SDMA
        # bumps the ring's sem +16 per call when the data has LANDED). The
        # scalar engine stays a pure tanh engine. Rings 1-3 reads are not
        # modeled by Tile, so ot-buffer reuse is guarded by slack wait_ge's
        # (16-deep pool, wait on the call issued 16 drains ago).
        kv_sem1 = nc.alloc_semaphore(name="kvsem1")
        kv_sem2 = nc.alloc_semaphore(name="kvsem2")
        kv_sem3 = nc.alloc_semaphore(name="kvsem3")
        kv_sems = {1: kv_sem1, 2: kv_sem2, 3: kv_sem3}
        kv_count = {1: 0, 2: 0, 3: 0}
        idx0 = singles.tile([128, 1], mybir.dt.int32)
        nc.vector.memset(idx0, 0)
        streams = [("dma", nc.sync), ("kv", 1), ("dma", nc.gpsimd),
                   ("kv", 2), ("kv", 3)]
        drain_log = []   # per drain: None (tracked dma) or (ring, ordinal)
        dma_i = 0

        def prep_pair(p):
            # rhs' for j-quads (2p, 2p+1): one mult + one add over [C, 8*V]
            tmp_p = tmp_pool.tile([C, 8 * V], _F32)
            ctxj_bc = bass.AP(
                tensor=ctxjT_sb.tensor,
                offset=ctxjT_sb.offset + 8 * p,
                ap=[ctxjT_sb.ap[0], [1, 8], [0, V]],
            )
            nc.vector.tensor_tensor(
                out=tmp_p, in0=wm_b8, in1=ctxj_bc, op=mybir.AluOpType.mult
            )
            rhs_p = rhs_pool.tile([C, 8 * V], _F16)
            nc.vector.tensor_tensor(
                out=rhs_p, in0=tmp_p, in1=w2d_b8, op=mybir.AluOpType.add
            )
            return rhs_p

        def pair_slice(pairs, bb):
            return pairs[bb // 2][:, (bb % 2) * 4 * V:(bb % 2 + 1) * 4 * V]

        def bias_mms(jb, half):
            # K=1 mms replicate arow across all 128 i-partitions, paid once
            # per jb thanks to the delta chain
            for s in range(4):
                bk = 4 * half + s
                q = 8 * jb + bk
                strip = (q % 4) * 32
                col = (q // 4) * 512
                nc.tensor.matmul(
                    P[:, bk * 512:(bk + 1) * 512],
                    lhsT=ones_sb[strip:strip + 1, :],
                    rhs=arowp_sb[strip:strip + 1, col:col + 512],
                    start=True,
                    stop=False,
                    tile_position=(strip, 0),
                )

        def drain(jb, lev, half, last):
            # tanh [128,2048] -> fp16; ship via the 5-stream rotation (one
            # 512 KiB transfer), 3-way HWDGE split on the final drain to cut
            # the tail
            nonlocal dma_i
            d = len(drain_log)
            if d >= 16 and drain_log[d - 16] is not None:
                ring, ordinal = drain_log[d - 16]
                nc.scalar.wait_ge(kv_sems[ring], 16 * (ordinal + 1))
            ot = out_pool.tile([128, 2048], _F16)
            nc.scalar.activation(
                ot, P[:, half * 2048:(half + 1) * 2048],
                mybir.ActivationFunctionType.Tanh,
            )
            base = (lev * 128) * NJ * V + (32 * jb + 16 * half) * V
            if last:
                for eng, p0, pn in ((nc.sync, 0, 48), (nc.gpsimd, 48, 48),
                                    (nc.scalar, 96, 32)):
                    dst = bass.AP(
                        tensor=out_d.tensor,
                        offset=base + p0 * NJ * V,
                        ap=[[NJ * V, pn], [1, 2048]],
                    )
                    eng.dma_start(out=dst, in_=ot[p0:p0 + pn, :])
                drain_log.append(None)
                return
            kind = streams[dma_i % len(streams)]
            dma_i += 1
            if kind[0] == "dma":
                dst = bass.AP(
                    tensor=out_d.tensor,
                    offset=base,
                    ap=[[NJ * V, 128], [1, 2048]],
                )
                kind[1].dma_start(out=dst, in_=ot)
                drain_log.append(None)
            else:
                ring = kind[1]
                kv_out = bass.AP(
                    tensor=out_d.tensor,
                    offset=base,
                    ap=[[NJ * V, 1], [NJ * V, 128], [NJ * V, 1], [1, 2048]],
                )
                kv_in = bass.AP(
                    tensor=ot.tensor,
                    offset=ot.offset,
                    ap=[ot.ap[0], [2048, 1], [2048, 1], [1, 2048]],
                )
                nc.gpsimd.kv_writeback(
                    out_ap=kv_out, in_ap=kv_in, ctx_idxs_ap=idx0[:, 0:1],
                    prepare_only=True, sem=kv_sems[ring], queue_num=ring,
                )
                nc.gpsimd.trigger_dma(count=None, queue_num=ring)
                drain_log.append((ring, kv_count[ring]))
                kv_count[ring] += 1

        for jb in range(NJB):
            if jb == 0:
                pairs = [prep_pair(0), prep_pair(1), None, None]
            else:
                pairs = [prep_pair(4 * jb + pp) for pp in range(4)]
                bias_mms(jb, 0)
                bias_mms(jb, 1)

            for lev in range(NLEV):
                lhsT = ctxiD_sb[:, lev * 128:(lev + 1) * 128]
                for half in range(2):
                    if jb == 0 and lev == 0:
                        # ramp: bias right before its half's mains; preps for
                        # pairs 1-3 trickle in behind pair 0
                        bias_mms(0, half)
                        if half == 1:
                            pairs[2] = prep_pair(2)
                            pairs[3] = prep_pair(3)
                    for s in range(4):
                        bk = 4 * half + s
                        nc.tensor.matmul(
                            P[:, bk * 512:(bk + 1) * 512],
                            lhsT=lhsT,
                            rhs=pair_slice(pairs, bk),
                            start=False,
                            stop=(lev == NLEV - 1),
                            skip_group_check=(lev > 0),
                        )
                    drain(jb, lev, half,
                          last=(jb == NJB - 1 and lev == NLEV - 1 and half == 1))

        # all kv-ring transfers must have landed before the NEFF can finish
        for ring, cnt in kv_count.items():
            if cnt:
                nc.gpsimd.wait_ge(kv_sems[ring], 16 * cnt)

    nc.compile()
    return nc


_NC_CACHE = {}


def get_nc():
    if "nc" not in _NC_CACHE:
        _NC_CACHE["nc"] = build_nc()
    return _NC_CACHE["nc"]


def make_in_maps(ctx, W1, b1, W2, b2, Wm, bm, Wd, bd):
    ctx = np.asarray(ctx, np.float32)
    bias_all = (
        np.asarray(b1) + np.asarray(b2) + np.asarray(bm) + np.asarray(bd)
    ).astype(np.float32)
    wmT = np.ascontiguousarray(np.asarray(Wm, np.float32).T)                   # (C,V)
    w2dT = np.ascontiguousarray(
        (np.asarray(W2) - np.asarray(Wd)).T.astype(np.float32)
    )
    aw = (np.asarray(W1) + np.asarray(Wd)).astype(np.float32)                  # (V,C)

    in_maps = []
    for k in range(NCORES):
        b = k // 2
        j0c = (k % 2) * NJ
        cT = ctx[b].T                                                          # (C,S)
        # level-0 chunk + successive deltas, rounded to fp16
        ctxiD = np.empty((C, S), np.float16)
        ctxiD[:, 0:128] = cT[:, 0:128]
        for lv in range(1, NLEV):
            ctxiD[:, lv * 128:(lv + 1) * 128] = (
                cT[:, lv * 128:(lv + 1) * 128] - cT[:, (lv - 1) * 128:lv * 128]
            )
        arow = (ctx[b, j0c:j0c + NJ] @ aw.T + bias_all).astype(np.float16)     # (NJ,V)
        arowq = arow.reshape(NQ, 4 * V)
        arowp = np.zeros((4, (NQ // 4) * 512), np.float16)
        for q in range(NQ):
            arowp[q % 4, (q // 4) * 512:(q // 4) * 512 + 512] = arowq[q]
        in_maps.append({
            "ctxjT": np.ascontiguousarray(cT[:, j0c:j0c + NJ]),
            "wmT": wmT,
            "w2dT": w2dT,
            "ctxiD": ctxiD,
            "arowp": arowp,
        })
    return in_maps


def run(in_maps, **kw):
    return run_bass_kernel_spmd(get_nc(), in_maps, core_ids=list(range(NCORES)), **kw)


def assemble(results):
    out = np.empty((B, S, S, V), np.float32)
    for k in range(NCORES):
        b = k // 2
        j0c = (k % 2) * NJ
        out[b, :, j0c:j0c + NJ, :] = results[k]["out_shard"]
    return out


def kernel(ctx, W1, b1, W2, b2, Wm, bm, Wd, bd):
    install_ntff_shim()
    in_maps = make_in_maps(ctx, W1, b1, W2, b2, Wm, bm, Wd, bd)
    res = run(in_maps)
    return assemble(res.results)


# revision 19
# speedup vs baseline: 1.1891x; 1.1891x over previous
"""TRN2 Bass kernel for nn_ComboFwdVecComp (B=4, S=512, C=V=128).

out[b,i,j,v] = tanh( sum_c ctx[b,i,c]*ctx[b,j,c]*Wm[v,c]            (M)
                     + ctx[b,i,:] @ (W2-Wd).T                       (Bi, i-dep)
                     + ctx[b,j,:] @ (W1+Wd).T + (b1+b2+bm+bd) )     (Aj, j-dep)

Sharding: core k handles b = k//2, j in [ (k%2)*256, +256 ), ALL 512 i's.
Shard out (S, 256, V) fp16; host casts to f32 and scatters on the j axis.

v2 design (vs the 292us f32 baseline):
- fp16 output store halves HBM write traffic (32 MiB/core); rel-err budget
  2e-2 >> fp16 ulp. PSUM partition dim = i so each drained half [128 i,
  16 j, 128 v] maps to (j,v)-contiguous HBM: 2-dim DMAs, 4 KiB runs.
- all matmuls in fp16 (1 cyc/row like f32r, tf32-class mantissa, no
  rounding-producer rule: host supplies fp16, DVE prep emits fp16).
- Delta-matmul chain kills the K=1 bias-replication redundancy: per
  (jblock, bank) PSUM accumulates  bias(arow) + ctx_i0 . rhs'  and is
  drained; then lhsT = fp16(ctx_ic{k} - ctx_ic{k-1}) matmuls morph it in
  place to chunk k's pre-tanh, re-drained, k = 1..3. Bias rows are paid
  once per 4 i-chunks: PE rows 8jb*8banks*(512 + 4*512) ~ 164k ~ 117us,
  below the ACT tanh floor (64 drains * ~1.97us = 126us).
- rhs'[c,(j,v)] = WmT[c,v]*ctx[b,j,c] + W2dT[c,v] folds Bi into the main
  matmul; Aj + biases ride the K=1 ones-matmul from strip-packed arow.
"""

import sys
import types
from contextlib import ExitStack

import numpy as np

import concourse.bass as bass
import concourse.mybir as mybir
import concourse.tile as tile
from concourse import bacc
from concourse.bass_utils import run_bass_kernel_spmd

B, S, C, V = 4, 512, 128, 128
NCORES = 8
NJ = 256          # j's per core
NJB = 8           # j-blocks of 32 j's
NQ = NJ // 4      # j-quads per core (64)
NLEV = 4          # i-chunks of 128 (level 0 + 3 deltas)

_F32 = mybir.dt.float32
_F16 = mybir.dt.float16


def install_ntff_shim():
    """antenv.axon_hooks is absent on some images; shim it so trace=True works."""
    if "antenv.axon_hooks" in sys.modules:
        return
    try:
        from trn_agent_boot.trn_boot import _ntff_profile_via_ctypes
        hook = _ntff_profile_via_ctypes("/opt/axon/libaxon_pjrt.so")
    except Exception:
        hook = None
    mod = types.ModuleType("antenv.axon_hooks")
    mod.get_axon_ntff_profile_hook = lambda: hook
    mod.set_axon_ntff_profile_hook = lambda h: None
    sys.modules["antenv.axon_hooks"] = mod


def build_nc():
    nc = bacc.Bacc("TRN2", target_bir_lowering=False, debug=False)

    ctxjT_d = nc.dram_tensor("ctxjT", [C, NJ], _F32, kind="ExternalInput").ap()
    wmT_d = nc.dram_tensor("wmT", [C, V], _F32, kind="ExternalInput").ap()
    w2dT_d = nc.dram_tensor("w2dT", [C, V], _F32, kind="ExternalInput").ap()
    # [fp16(ctxT i 0:128) | delta1 | delta2 | delta3]
    ctxiD_d = nc.dram_tensor("ctxiD", [C, S], _F16, kind="ExternalInput").ap()
    # arow rows, strip-packed: quad q -> partition (q%4)*32, cols (q//4)*512
    arowp_d = nc.dram_tensor("arowp", [4, (NQ // 4) * 512], _F16, kind="ExternalInput").ap()
    out_d = nc.dram_tensor("out_shard", [S, NJ, V], _F16, kind="ExternalOutput").ap()

    with tile.TileContext(nc) as tc, ExitStack() as ctx:
        singles = ctx.enter_context(tc.tile_pool(name="singles", bufs=1))
        rhs_pool = ctx.enter_context(tc.tile_pool(name="rhs", bufs=8))
        tmp_pool = ctx.enter_context(tc.tile_pool(name="tmp", bufs=3))
        psum_pool = ctx.enter_context(tc.tile_pool(name="psum", bufs=1, space="PSUM"))
        out_pool = ctx.enter_context(tc.tile_pool(name="outs", bufs=12))

        # ---- load constants; arowp rows gate the first bias mms -> FIRST ----
        arowp_sb = singles.tile([97, (NQ // 4) * 512], _F16)
        for r in range(4):
            eng = nc.sync if r % 2 == 0 else nc.scalar
            eng.dma_start(out=arowp_sb[32 * r:32 * r + 1, :], in_=arowp_d[r:r + 1, :])
        ctxiD_sb = singles.tile([C, S], _F16)
        nc.sync.dma_start(out=ctxiD_sb, in_=ctxiD_d)
        # first prep pair only needs ctxjT cols 0:8 -> tiny chunk first
        ctxjT_sb = singles.tile([C, NJ], _F32)
        nc.scalar.dma_start(out=ctxjT_sb[:, 0:8], in_=ctxjT_d[:, 0:8])
        wmT_sb = singles.tile([C, V], _F32)
        nc.scalar.dma_start(out=wmT_sb, in_=wmT_d)
        w2dT_sb = singles.tile([C, V], _F32)
        nc.scalar.dma_start(out=w2dT_sb, in_=w2dT_d)
        nc.scalar.dma_start(out=ctxjT_sb[:, 8:NJ], in_=ctxjT_d[:, 8:NJ])

        ones_sb = singles.tile([97, 128], _F16)
        nc.vector.memset(ones_sb, 1.0)

        # broadcast APs for pair-wide (8 j's) prep: wmT/w2dT repeat over the
        # j dim (step 0), ctxj scalars repeat over the v dim (trailing step 0)
        wm_b8 = bass.AP(
            tensor=wmT_sb.tensor,
            offset=wmT_sb.offset,
            ap=[wmT_sb.ap[0], [0, 8], wmT_sb.ap[1]],
        )
        w2d_b8 = bass.AP(
            tensor=w2dT_sb.tensor,
            offset=w2dT_sb.offset,
            ap=[w2dT_sb.ap[0], [0, 8], w2dT_sb.ap[1]],
        )

        # one 8-bank psum megatile; bank b occupies [:, b*512:(b+1)*512]
        P = psum_pool.tile([128, 4096], _F32, name="mega")

        # per-core writes cap at ~250 GB/s aggregate regardless of stream
        # count (kv_writeback on SWDGE rings 1-3 was tried: per-ring rate
        # collapsed to ~54 GB/s and Pool desc-gen serialized at 130us), so
        # 3 queues is the right shape.
        dma_engines = [nc.sync, nc.gpsimd, nc.scalar]
        dma_i = 0

        def prep_pair(p):
            # rhs' for j-quads (2p, 2p+1): one mult + one add over [C, 8*V]
            tmp_p = tmp_pool.tile([C, 8 * V], _F32)
            ctxj_bc = bass.AP(
                tensor=ctxjT_sb.tensor,
                offset=ctxjT_sb.offset + 8 * p,
                ap=[ctxjT_sb.ap[0], [1, 8], [0, V]],
            )
            nc.vector.tensor_tensor(
                out=tmp_p, in0=wm_b8, in1=ctxj_bc, op=mybir.AluOpType.mult
            )
            rhs_p = rhs_pool.tile([C, 8 * V], _F16)
            nc.vector.tensor_tensor(
                out=rhs_p, in0=tmp_p, in1=w2d_b8, op=mybir.AluOpType.add
            )
            return rhs_p

        def pair_slice(pairs, bb):
            return pairs[bb // 2][:, (bb % 2) * 4 * V:(bb % 2 + 1) * 4 * V]

        def bias_mms(jb, half):
            # K=1 mms replicate arow across all 128 i-partitions, paid once
            # per jb thanks to the delta chain
            for s in range(4):
                bk = 4 * half + s
                q = 8 * jb + bk
                strip = (q % 4) * 32
                col = (q // 4) * 512
                nc.tensor.matmul(
                    P[:, bk * 512:(bk + 1) * 512],
                    lhsT=ones_sb[strip:strip + 1, :],
                    rhs=arowp_sb[strip:strip + 1, col:col + 512],
                    start=True,
                    stop=False,
                    tile_position=(strip, 0),
                )

        def drain(jb, lev, half, last):
            # tanh [128,2048] -> fp16; one 512 KiB DMA (3-way split on the
            # final drain to cut the single-queue tail)
            nonlocal dma_i
            ot = out_pool.tile([128, 2048], _F16)
            nc.scalar.activation(
                ot, P[:, half * 2048:(half + 1) * 2048],
                mybir.ActivationFunctionType.Tanh,
            )
            base = (lev * 128) * NJ * V + (32 * jb + 16 * half) * V
            splits = ((0, 48), (48, 48), (96, 32)) if last else ((0, 128),)
            for p0, pn in splits:
                dst = bass.AP(
                    tensor=out_d.tensor,
                    offset=base + p0 * NJ * V,
                    ap=[[NJ * V, pn], [1, 2048]],
                )
                eng = dma_engines[dma_i % 3]
                dma_i += 1
                eng.dma_start(out=dst, in_=ot[p0:p0 + pn, :])

        for jb in range(NJB):
            if jb == 0:
                pairs = [prep_pair(0), prep_pair(1), None, None]
            else:
                pairs = [prep_pair(4 * jb + pp) for pp in range(4)]
                bias_mms(jb, 0)
                bias_mms(jb, 1)

            for lev in range(NLEV):
                lhsT = ctxiD_sb[:, lev * 128:(lev + 1) * 128]
                for half in range(2):
                    if jb == 0 and lev == 0:
                        # ramp: bias right before its half's mains; preps for
                        # pairs 1-3 trickle in behind pair 0
                        bias_mms(0, half)
                        if half == 1:
                            pairs[2] = prep_pair(2)
                            pairs[3] = prep_pair(3)
                    for s in range(4):
                        bk = 4 * half + s
                        nc.tensor.matmul(
                            P[:, bk * 512:(bk + 1) * 512],
                            lhsT=lhsT,
                            rhs=pair_slice(pairs, bk),
                            start=False,
                            stop=(lev == NLEV - 1),
                            skip_group_check=(lev > 0),
                        )
                    drain(jb, lev, half,
                          last=(jb == NJB - 1 and lev == NLEV - 1 and half == 1))

    nc.compile()
    return nc


_NC_CACHE = {}


def get_nc():
    if "nc" not in _NC_CACHE:
        _NC_CACHE["nc"] = build_nc()
    return _NC_CACHE["nc"]


def make_in_maps(ctx, W1, b1, W2, b2, Wm, bm, Wd, bd):
    ctx = np.asarray(ctx, np.float32)
    bias_all = (
        np.asarray(b1) + np.asarray(b2) + np.asarray(bm) + np.asarray(bd)
    ).astype(np.float32)
    wmT = np.ascontiguousarray(np.asarray(Wm, np.float32).T)                   # (C,V)
    w2dT = np.ascontiguousarray(
        (np.asarray(W2) - np.asarray(Wd)).T.astype(np.float32)
    )
    aw = (np.asarray(W1) + np.asarray(Wd)).astype(np.float32)                  # (V,C)

    in_maps = []
    for k in range(NCORES):
        b = k // 2
        j0c = (k % 2) * NJ
        cT = ctx[b].T                                                          # (C,S)
        # level-0 chunk + successive deltas, rounded to fp16
        ctxiD = np.empty((C, S), np.float16)
        ctxiD[:, 0:128] = cT[:, 0:128]
        for lv in range(1, NLEV):
            ctxiD[:, lv * 128:(lv + 1) * 128] = (
                cT[:, lv * 128:(lv + 1) * 128] - cT[:, (lv - 1) * 128:lv * 128]
            )
        arow = (ctx[b, j0c:j0c + NJ] @ aw.T + bias_all).astype(np.float16)     # (NJ,V)
        arowq = arow.reshape(NQ, 4 * V)
        arowp = np.zeros((4, (NQ // 4) * 512), np.float16)
        for q in range(NQ):
            arowp[q % 4, (q // 4) * 512:(q // 4) * 512 + 512] = arowq[q]
        in_maps.append({
            "ctxjT": np.ascontiguousarray(cT[:, j0c:j0c + NJ]),
            "wmT": wmT,
            "w2dT": w2dT,
            "ctxiD": ctxiD,
            "arowp": arowp,
        })
    return in_maps


def run(in_maps, **kw):
    return run_bass_kernel_spmd(get_nc(), in_maps, core_ids=list(range(NCORES)), **kw)


def assemble(results):
    out = np.empty((B, S, S, V), np.float32)
    for k in range(NCORES):
        b = k // 2
        j0c = (k % 2) * NJ
        out[b, :, j0c:j0c + NJ, :] = results[k]["out_shard"]
    return out


def kernel(ctx, W1, b1, W2, b2, Wm, bm, Wd, bd):
    install_ntff_shim()
    in_maps = make_in_maps(ctx, W1, b1, W2, b2, Wm, bm, Wd, bd)
    res = run(in_maps)
    return assemble(res.results)


# revision 21
# speedup vs baseline: 1.1968x; 1.0065x over previous
"""TRN2 Bass kernel for nn_ComboFwdVecComp (B=4, S=512, C=V=128).

out[b,i,j,v] = tanh( sum_c ctx[b,i,c]*ctx[b,j,c]*Wm[v,c]            (M)
                     + ctx[b,i,:] @ (W2-Wd).T                       (Bi, i-dep)
                     + ctx[b,j,:] @ (W1+Wd).T + (b1+b2+bm+bd) )     (Aj, j-dep)

Sharding: core k handles b = k//2, j in [ (k%2)*256, +256 ), ALL 512 i's.
Shard out (S, 256, V) fp16; host casts to f32 and scatters on the j axis.

v2 design (vs the 292us f32 baseline):
- fp16 output store halves HBM write traffic (32 MiB/core); rel-err budget
  2e-2 >> fp16 ulp. PSUM partition dim = i so each drained half [128 i,
  16 j, 128 v] maps to (j,v)-contiguous HBM: 2-dim DMAs, 4 KiB runs.
- all matmuls in fp16 (1 cyc/row like f32r, tf32-class mantissa, no
  rounding-producer rule: host supplies fp16, DVE prep emits fp16).
- Delta-matmul chain kills the K=1 bias-replication redundancy: per
  (jblock, bank) PSUM accumulates  bias(arow) + ctx_i0 . rhs'  and is
  drained; then lhsT = fp16(ctx_ic{k} - ctx_ic{k-1}) matmuls morph it in
  place to chunk k's pre-tanh, re-drained, k = 1..3. Bias rows are paid
  once per 4 i-chunks: PE rows 8jb*8banks*(512 + 4*512) ~ 164k ~ 117us,
  below the ACT tanh floor (64 drains * ~1.97us = 126us).
- rhs'[c,(j,v)] = WmT[c,v]*ctx[b,j,c] + W2dT[c,v] folds Bi into the main
  matmul; Aj + biases ride the K=1 ones-matmul from strip-packed arow.
"""

import sys
import types
from contextlib import ExitStack

import numpy as np

import concourse.bass as bass
import concourse.mybir as mybir
import concourse.tile as tile
from concourse import bacc
from concourse.bass_utils import run_bass_kernel_spmd

B, S, C, V = 4, 512, 128, 128
NCORES = 8
NJ = 256          # j's per core
NJB = 8           # j-blocks of 32 j's
NQ = NJ // 4      # j-quads per core (64)
NLEV = 4          # i-chunks of 128 (level 0 + 3 deltas)

_F32 = mybir.dt.float32
_F16 = mybir.dt.float16


def install_ntff_shim():
    """antenv.axon_hooks is absent on some images; shim it so trace=True works."""
    if "antenv.axon_hooks" in sys.modules:
        return
    try:
        from trn_agent_boot.trn_boot import _ntff_profile_via_ctypes
        hook = _ntff_profile_via_ctypes("/opt/axon/libaxon_pjrt.so")
    except Exception:
        hook = None
    mod = types.ModuleType("antenv.axon_hooks")
    mod.get_axon_ntff_profile_hook = lambda: hook
    mod.set_axon_ntff_profile_hook = lambda h: None
    sys.modules["antenv.axon_hooks"] = mod


def build_nc():
    nc = bacc.Bacc("TRN2", target_bir_lowering=False, debug=False)

    ctxjT_d = nc.dram_tensor("ctxjT", [C, NJ], _F32, kind="ExternalInput").ap()
    wmT_d = nc.dram_tensor("wmT", [C, V], _F32, kind="ExternalInput").ap()
    w2dT_d = nc.dram_tensor("w2dT", [C, V], _F32, kind="ExternalInput").ap()
    # [fp16(ctxT i 0:128) | delta1 | delta2 | delta3]
    ctxiD_d = nc.dram_tensor("ctxiD", [C, S], _F16, kind="ExternalInput").ap()
    # arow rows, strip-packed: quad q -> partition (q%4)*32, cols (q//4)*512
    arowp_d = nc.dram_tensor("arowp", [4, (NQ // 4) * 512], _F16, kind="ExternalInput").ap()
    out_d = nc.dram_tensor("out_shard", [S, NJ, V], _F16, kind="ExternalOutput").ap()

    with tile.TileContext(nc) as tc, ExitStack() as ctx:
        singles = ctx.enter_context(tc.tile_pool(name="singles", bufs=1))
        rhs_pool = ctx.enter_context(tc.tile_pool(name="rhs", bufs=8))
        tmp_pool = ctx.enter_context(tc.tile_pool(name="tmp", bufs=3))
        psum_pool = ctx.enter_context(tc.tile_pool(name="psum", bufs=1, space="PSUM"))
        out_pool = ctx.enter_context(tc.tile_pool(name="outs", bufs=8))

        # ---- load constants; arowp rows gate the first bias mms -> FIRST ----
        arowp_sb = singles.tile([97, (NQ // 4) * 512], _F16)
        for r in range(4):
            eng = nc.sync if r % 2 == 0 else nc.scalar
            eng.dma_start(out=arowp_sb[32 * r:32 * r + 1, :], in_=arowp_d[r:r + 1, :])
        ctxiD_sb = singles.tile([C, S], _F16)
        nc.sync.dma_start(out=ctxiD_sb, in_=ctxiD_d)
        ctxjT_sb = singles.tile([C, NJ], _F32)
        nc.scalar.dma_start(out=ctxjT_sb, in_=ctxjT_d)
        wmT_sb = singles.tile([C, V], _F32)
        nc.scalar.dma_start(out=wmT_sb, in_=wmT_d)
        w2dT_sb = singles.tile([C, V], _F32)
        nc.scalar.dma_start(out=w2dT_sb, in_=w2dT_d)

        ones_sb = singles.tile([97, 128], _F16)
        nc.vector.memset(ones_sb, 1.0)

        # broadcast APs for pair-wide (8 j's) prep: wmT/w2dT repeat over the
        # j dim (step 0), ctxj scalars repeat over the v dim (trailing step 0)
        wm_b8 = bass.AP(
            tensor=wmT_sb.tensor,
            offset=wmT_sb.offset,
            ap=[wmT_sb.ap[0], [0, 8], wmT_sb.ap[1]],
        )
        w2d_b8 = bass.AP(
            tensor=w2dT_sb.tensor,
            offset=w2dT_sb.offset,
            ap=[w2dT_sb.ap[0], [0, 8], w2dT_sb.ap[1]],
        )

        # one 8-bank psum megatile; bank b occupies [:, b*512:(b+1)*512]
        P = psum_pool.tile([128, 4096], _F32, name="mega")

        # per-core writes cap at ~250 GB/s aggregate regardless of stream
        # count (kv_writeback on SWDGE rings 1-3 was tried: per-ring rate
        # collapsed to ~54 GB/s and Pool desc-gen serialized at 130us), so
        # 3 queues is the right shape.
        dma_engines = [nc.sync, nc.gpsimd, nc.scalar]
        dma_i = 0

        def prep_pair(p):
            # rhs' for j-quads (2p, 2p+1): one mult + one add over [C, 8*V]
            tmp_p = tmp_pool.tile([C, 8 * V], _F32)
            ctxj_bc = bass.AP(
                tensor=ctxjT_sb.tensor,
                offset=ctxjT_sb.offset + 8 * p,
                ap=[ctxjT_sb.ap[0], [1, 8], [0, V]],
            )
            nc.vector.tensor_tensor(
                out=tmp_p, in0=wm_b8, in1=ctxj_bc, op=mybir.AluOpType.mult
            )
            rhs_p = rhs_pool.tile([C, 8 * V], _F16)
            nc.vector.tensor_tensor(
                out=rhs_p, in0=tmp_p, in1=w2d_b8, op=mybir.AluOpType.add
            )
            return rhs_p

        def pair_slice(pairs, bb):
            return pairs[bb // 2][:, (bb % 2) * 4 * V:(bb % 2 + 1) * 4 * V]

        def bias_mms(jb, half):
            # K=1 mms replicate arow across all 128 i-partitions, paid once
            # per jb thanks to the delta chain
            for s in range(4):
                bk = 4 * half + s
                q = 8 * jb + bk
                strip = (q % 4) * 32
                col = (q // 4) * 512
                nc.tensor.matmul(
                    P[:, bk * 512:(bk + 1) * 512],
                    lhsT=ones_sb[strip:strip + 1, :],
                    rhs=arowp_sb[strip:strip + 1, col:col + 512],
                    start=True,
                    stop=False,
                    tile_position=(strip, 0),
                )

        def drain(jb, lev, half, last):
            # tanh [128,2048] -> fp16; one 512 KiB DMA (3-way split on the
            # final drain to cut the single-queue tail)
            nonlocal dma_i
            ot = out_pool.tile([128, 2048], _F16)
            nc.scalar.activation(
                ot, P[:, half * 2048:(half + 1) * 2048],
                mybir.ActivationFunctionType.Tanh,
            )
            base = (lev * 128) * NJ * V + (32 * jb + 16 * half) * V
            splits = ((0, 48), (48, 48), (96, 32)) if last else ((0, 128),)
            for p0, pn in splits:
                dst = bass.AP(
                    tensor=out_d.tensor,
                    offset=base + p0 * NJ * V,
                    ap=[[NJ * V, pn], [1, 2048]],
                )
                eng = dma_engines[dma_i % 3]
                dma_i += 1
                eng.dma_start(out=dst, in_=ot[p0:p0 + pn, :])

        for jb in range(NJB):
            if jb == 0:
                pairs = [prep_pair(0), prep_pair(1), None, None]
            else:
                pairs = [prep_pair(4 * jb + pp) for pp in range(4)]
                bias_mms(jb, 0)
                bias_mms(jb, 1)

            for lev in range(NLEV):
                lhsT = ctxiD_sb[:, lev * 128:(lev + 1) * 128]
                for half in range(2):
                    if jb == 0 and lev == 0:
                        # ramp: bias right before its half's mains; preps for
                        # pairs 1-3 trickle in behind pair 0
                        bias_mms(0, half)
                        if half == 1:
                            pairs[2] = prep_pair(2)
                            pairs[3] = prep_pair(3)
                    for s in range(4):
                        bk = 4 * half + s
                        nc.tensor.matmul(
                            P[:, bk * 512:(bk + 1) * 512],
                            lhsT=lhsT,
                            rhs=pair_slice(pairs, bk),
                            start=False,
                            stop=(lev == NLEV - 1),
                            skip_group_check=(lev > 0),
                        )
                    drain(jb, lev, half,
                          last=(jb == NJB - 1 and lev == NLEV - 1 and half == 1))

    nc.compile()
    return nc


_NC_CACHE = {}


def get_nc():
    if "nc" not in _NC_CACHE:
        _NC_CACHE["nc"] = build_nc()
    return _NC_CACHE["nc"]


def make_in_maps(ctx, W1, b1, W2, b2, Wm, bm, Wd, bd):
    ctx = np.asarray(ctx, np.float32)
    bias_all = (
        np.asarray(b1) + np.asarray(b2) + np.asarray(bm) + np.asarray(bd)
    ).astype(np.float32)
    wmT = np.ascontiguousarray(np.asarray(Wm, np.float32).T)                   # (C,V)
    w2dT = np.ascontiguousarray(
        (np.asarray(W2) - np.asarray(Wd)).T.astype(np.float32)
    )
    aw = (np.asarray(W1) + np.asarray(Wd)).astype(np.float32)                  # (V,C)

    in_maps = []
    for k in range(NCORES):
        b = k // 2
        j0c = (k % 2) * NJ
        cT = ctx[b].T                                                          # (C,S)
        # level-0 chunk + successive deltas, rounded to fp16
        ctxiD = np.empty((C, S), np.float16)
        ctxiD[:, 0:128] = cT[:, 0:128]
        for lv in range(1, NLEV):
            ctxiD[:, lv * 128:(lv + 1) * 128] = (
                cT[:, lv * 128:(lv + 1) * 128] - cT[:, (lv - 1) * 128:lv * 128]
            )
        arow = (ctx[b, j0c:j0c + NJ] @ aw.T + bias_all).astype(np.float16)     # (NJ,V)
        arowq = arow.reshape(NQ, 4 * V)
        arowp = np.zeros((4, (NQ // 4) * 512), np.float16)
        for q in range(NQ):
            arowp[q % 4, (q // 4) * 512:(q // 4) * 512 + 512] = arowq[q]
        in_maps.append({
            "ctxjT": np.ascontiguousarray(cT[:, j0c:j0c + NJ]),
            "wmT": wmT,
            "w2dT": w2dT,
            "ctxiD": ctxiD,
            "arowp": arowp,
        })
    return in_maps


def run(in_maps, **kw):
    return run_bass_kernel_spmd(get_nc(), in_maps, core_ids=list(range(NCORES)), **kw)


def assemble(results):
    out = np.empty((B, S, S, V), np.float32)
    for k in range(NCORES):
        b = k // 2
        j0c = (k % 2) * NJ
        out[b, :, j0c:j0c + NJ, :] = results[k]["out_shard"]
    return out


def kernel(ctx, W1, b1, W2, b2, Wm, bm, Wd, bd):
    install_ntff_shim()
    in_maps = make_in_maps(ctx, W1, b1, W2, b2, Wm, bm, Wd, bd)
    res = run(in_maps)
    return assemble(res.results)
